# revision 39
# baseline (speedup 1.0000x reference)
"""Trainium2 Bass kernel for the ExemplarBaseline retrieval-kNN model.

Math (per batch b, fully independent across b):
    f      = data.reshape(B*T, CHW) @ W_fe + b_fe            (feature extract)
    d2     = ||f_s - f_t||^2 ; dist = d2**0.25
    sims   = exp(-c * dist)
    numers = 1e-8 + sum_{s<t} sims[s,t] * teach[s, cls]
    score  = numers**gamma / sum_cls ; score[t=0] = 1e-8

Sharding: data-parallel over the batch dim B (128) across 8 NeuronCores,
16 sequences per core (BL=16, T=128 -> TOK=2048 tokens per core).

Structure (v4):
  - Host pre-permutes x/W/teach so every DMA descriptor is a 1.5KB+
    contiguous run; ALL inputs (x 48KB/partition fp8 + W 24KB) live in
    SBUF, DMA'd up front in pieces ordered by DEADLINE across BOTH
    HWDGE trigger engines (sync + scalar) so the critical ~2MB for the
    first chunk's dt0 drains first.
  - feats^T = W^T x^T in fp8+DoubleRow (2x PE rate), evacuated with
    bias-add directly to fp8 fT pair tiles [128, 2, TOK].
  - d2 = sq_s + sq_t - 2*Gram, built ENTIRELY inside one PSUM group per
    sequence: 4 fp8-DoubleRow Gram matmuls, then sq is read off the
    Gram DIAGONAL (DVE copy to bf16, gpsimd affine_select is_equal ->
    diag tile) and both rank-1 corrections are two bf16 matmuls against
    a -0.5 constant tile (stat=diag/mov=-.5 adds -0.5*sq_s;
    stat=-.5/mov=diag adds -0.5*sq_t).  This kills the old per-chunk sq
    machinery (f2 muls on DVE, ones-matmul on PE, serial hi/lo fp8
    chain) and is MORE precise (bf16 sq vs fp8 hi/lo pair).
  - sims = exp(-c * exp(0.25 * ln(-2*pg))) -- Ln/Exp share one ACT
    table set (patched chooser), no table reloads anywhere; c is baked
    into the compiled kernel as an immediate scale (AP-scale ACTs cost
    ~90ns extra each).
  - gamma == 1 fast path (the reference setup fills gamma with ones):
    score = (numers+eps)/sum(numers+eps) on DVE only -- no Ln/Exp in
    the class normalizer.  A general-gamma variant is compiled instead
    when the host sees gamma != 1.
  - the triangular mask runs on DIST (gpsimd affine_select, fill big
    enough that exp(-c*fill)==0), so the final Exp writes the masked
    bf16 sims directly and gpsimd is off the numers critical path.
  - token chunks 4 x 512; epilogue split epiA1 (Gram+diag) / epiA2
    (rank1s + ACT chain) / epiB (numers+norm) spread through the NEXT
    chunk's d-tile slots; the LAST chunk's dt7 is evacuated per
    sequence and its ladders issue ACT-first (the tail is ACT-bound,
    so the PE waits on the diag extract instead of the reverse).
  - scores staged in one persistent [128, 160] f32 tile (partition=t),
    ONE output DMA; the host does the tiny [T,BL,NC]->[BL,T,NC]
    transpose and the t==0 EPS override.
Error budget: rel err ~9.5e-3 measured vs the 2e-2 gate (fp8 feats
dominates; fp8 Gram + bf16 sq/sims/teach add the rest).
"""

import numpy as np
import ml_dtypes

B, T, NC = 128, 128, 10
CHW, D = 3072, 1024
NCORES = 8
BL = B // NCORES          # 16 sequences per core
TOK = BL * T              # 2048 tokens per core
KT = CHW // 128           # 24 contraction tiles
DT = D // 128             # 8 feature tiles

CHUNKS = [512, 512, 512, 512]          # token columns per chunk
NSEQ = [w // T for w in CHUNKS]        # sequences per chunk [4,4,4,4]
C0 = [sum(CHUNKS[:i]) for i in range(len(CHUNKS))]   # chunk col starts

EPS_NUMER = 1e-8

_NC_CACHE = {}
LAST_RESULTS = None       # BassKernelResults of the most recent run (for test.py)


def _build_bass(gamma_is_one, cval):
    import concourse.mybir as mybir
    import concourse.tile as tile
    from concourse import bacc

    f32 = mybir.dt.float32
    bf16 = mybir.dt.bfloat16
    fp8 = mybir.dt.float8e4
    AF = mybir.ActivationFunctionType
    OP = mybir.AluOpType
    PM = mybir.MatmulPerfMode

    # The ACT table-set chooser picks the FIRST set containing each function:
    # Exp -> set 0, Ln -> set 5, which makes every Ln<->Exp transition reload
    # tables (~1.3us each).  Both live together in natural_log_exp_and_others;
    # hide them from every other set so the chooser lands there once.
    if not getattr(bacc, "_ln_exp_tables_patched", False):
        orig_tables = bacc.get_activation_tables

        def _patched_tables(arch):
            out = {}
            for name, funcs in orig_tables(arch).items():
                if name != "natural_log_exp_and_others":
                    funcs = funcs - {AF.Ln, AF.Exp}
                out[name] = funcs
            return out

        bacc.get_activation_tables = _patched_tables
        bacc._ln_exp_tables_patched = True

    nc = bacc.Bacc("TRN2", target_bir_lowering=False)

    # Host-side layouts (see make_in_maps): per-chunk x tensors and
    # dt-major W so every DMA slice is contiguous per partition.
    x_h = [
        nc.dram_tensor(f"xh{c}", [128, KT * w], fp8, kind="ExternalInput")
        for c, w in enumerate(CHUNKS)
    ]
    W_h = nc.dram_tensor("Wh", [128, DT * KT * 128], fp8, kind="ExternalInput")
    teach_h = nc.dram_tensor("teach", [T, BL * NC], bf16, kind="ExternalInput")
    pars_h = nc.dram_tensor("pars", [128, 2 + DT], f32, kind="ExternalInput")
    y_h = nc.dram_tensor("yT2", [T, BL * NC], f32, kind="ExternalOutput")

    with tile.TileContext(nc) as tc:
        with (
            tc.tile_pool(name="cpool", bufs=1) as cpool,
            tc.tile_pool(name="wpool", bufs=4) as wpool,
            tc.tile_pool(name="gcpool", bufs=2) as gcpool,
            tc.tile_pool(name="dtlpool", bufs=3) as dtlpool,
            tc.tile_pool(name="smpool", bufs=3) as smpool,
            tc.tile_pool(name="spool", bufs=6) as spool,
            tc.tile_pool(name="pfpool", bufs=2, space="PSUM") as pfpool,
            tc.tile_pool(name="pgpool", bufs=3, space="PSUM") as pgpool,
            tc.tile_pool(name="pnpool", bufs=1, space="PSUM") as pnpool,
        ):
            # ---- persistent tiles -------------------------------------
            W_sb = cpool.tile([128, DT, KT, 128], fp8, name="W_sb")
            x_sb = [
                cpool.tile([128, KT, w], fp8, name=f"x_sb{c}")
                for c, w in enumerate(CHUNKS)
            ]
            teach_sb = cpool.tile([128, BL, NC], bf16, name="teach_sb")
            pars_sb = cpool.tile([128, 2 + DT], f32, name="pars_sb")
            eps_sb = cpool.tile([128, 1], f32, name="eps_sb")
            # all scores staged here (partition = t), ONE output DMA at
            # the end; the host does the final [T,BL,NC]->[BL,T,NC]
            # transpose and the t==0 EPS override
            scb_all = cpool.tile([128, BL * NC], f32, name="scb_all")
            # fT in fp8 DoubleRow pair layout: tile p holds d-tiles 2p, 2p+1
            fTp = [
                cpool.tile([128, 2, TOK], fp8, name=f"fTp{i}")
                for i in range(DT // 2)
            ]
            # constant -0.5 plane for the rank-1 sq corrections
            neghalf = cpool.tile([128, 128], bf16, name="neghalf")
            gam = pars_sb[:, 1:2]

            # ---- all input DMAs, deadline order -----------------------
            # Each dma_start costs ~0.62us of DIRECT2D descriptor-writing
            # on its issuing sequencer and each sequencer's ring holds only
            # 4 in-flight pieces, so the critical pieces (x0 + W dt0, the
            # ~2MB chunk-0/dt0 working set) lead BOTH trigger engines.
            def xpiece(eng, c, k0, k1):
                eng.dma_start(
                    out=x_sb[c][:, k0:k1, :],
                    in_=x_h[c][:, k0 * CHUNKS[c]:k1 * CHUNKS[c]],
                )

            def wpiece(eng, dt_i, k0, k1):
                KW = KT * 128
                eng.dma_start(
                    out=W_sb[:, dt_i, k0:k1, :],
                    in_=W_h[:, dt_i * KW + k0 * 128:dt_i * KW + k1 * 128],
                )

            # sync: x0 k0:12 in 4 pieces, then W dt2..7, teach, x2
            for k in range(0, 12, 3):                  # x0 p1-p4
                xpiece(nc.sync, 0, k, k + 3)
            for dt_i in range(2, DT):                  # W dt2..7
                wpiece(nc.sync, dt_i, 0, KT)
            nc.sync.dma_start(out=teach_sb, in_=teach_h[:, :])
            for k in range(0, KT, 6):                  # x2: 4 pieces
                xpiece(nc.sync, 2, k, k + 6)
            # scalar (ACT): W dt0 + x0 tail interleaved, pars early (it
            # gates the very first feats evacuation), then W dt1, x1, x3
            wpiece(nc.scalar, 0, 0, 12)                # W dt0 p1
            xpiece(nc.scalar, 0, 12, 18)               # x0 p5
            wpiece(nc.scalar, 0, 12, KT)               # W dt0 p2
            nc.scalar.dma_start(out=pars_sb, in_=pars_h[:, :])
            xpiece(nc.scalar, 0, 18, KT)               # x0 p6
            wpiece(nc.scalar, 1, 0, 12)                # W dt1 p1
            wpiece(nc.scalar, 1, 12, KT)               # W dt1 p2
            for k in range(0, KT, 6):                  # x1: 4 pieces
                xpiece(nc.scalar, 1, k, k + 6)
            for k in range(0, KT, 6):                  # x3: 4 pieces
                xpiece(nc.scalar, 3, k, k + 6)

            # constants: few-partition memsets are slow on DVE,
            # gpsimd is idle at startup
            nc.gpsimd.memset(neghalf, -0.5)
            nc.vector.memset(eps_sb, EPS_NUMER)

            # ---- per-(chunk, d-tile) feats ----------------------------
            def feats_dt(c, dt_i):
                w = CHUNKS[c]
                csl = slice(C0[c], C0[c] + w)
                pf = pfpool.tile([128, w], f32, name="pf")
                for k in range(0, KT, 2):
                    nc.tensor.matmul(
                        pf, W_sb[:, dt_i, k:k + 2, :], x_sb[c][:, k:k + 2, :],
                        start=(k == 0), stop=(k == KT - 2),
                        perf_mode=PM.DoubleRow,
                    )
                # evacuate psum -> fp8 fT pair tile with per-partition bias
                # add.  On DVE so the scalar engine only ever runs Ln/Exp.
                pair = fTp[dt_i // 2][:, dt_i % 2, :]
                if c == len(CHUNKS) - 1 and dt_i == DT - 1:
                    # last chunk's last dt: evacuate per SEQUENCE so each
                    # Gram group starts after only its own 128 columns.
                    for i in range(NSEQ[c]):
                        nc.vector.tensor_scalar(
                            pair[:, C0[c] + i * T:C0[c] + (i + 1) * T],
                            pf[:, i * T:(i + 1) * T],
                            pars_sb[:, 2 + dt_i:3 + dt_i], None, op0=OP.add,
                        )
                else:
                    nc.vector.tensor_scalar(
                        pair[:, csl], pf,
                        pars_sb[:, 2 + dt_i:3 + dt_i], None, op0=OP.add,
                    )

            # ---- per-sequence epilogue, split A1/A2/B ------------------
            pg_of, dtl_of, sims_of = {}, {}, {}

            def epiA1(b):
                # Gram accumulation (group left OPEN) + diagonal extract:
                # diag(G)_ss = sq_s, pulled by gpsimd off an SBUF copy.
                tsl = slice(b * T, (b + 1) * T)
                pg = pgpool.tile([128, 128], f32, name="pg")
                for p in range(DT // 2):
                    nc.tensor.matmul(
                        pg, fTp[p][:, :, tsl], fTp[p][:, :, tsl],
                        start=(p == 0), stop=False, perf_mode=PM.DoubleRow,
                    )
                gcp = gcpool.tile([128, 128], bf16, name="gcp")
                nc.vector.tensor_copy(gcp, pg)
                dtl = dtlpool.tile([128, 128], bf16, name="dtl")
                nc.gpsimd.affine_select(
                    out=dtl, in_=gcp,
                    compare_op=OP.is_equal, fill=0.0,
                    base=0, pattern=[[1, 128]], channel_multiplier=-1,
                )
                pg_of[b], dtl_of[b] = pg, dtl

            def epiA2(b):
                # rank-1 corrections off the diag tile, then the ACT chain.
                pg, dtl = pg_of.pop(b), dtl_of.pop(b)
                # stat=diag, mov=-0.5 plane: pg[s,t] += -0.5*sq_s
                nc.tensor.matmul(pg, dtl, neghalf, start=False, stop=False)
                # stat=-0.5 plane, mov=diag: pg[s,t] += -0.5*sq_t
                nc.tensor.matmul(pg, neghalf, dtl, start=False, stop=True)
                # dist = exp(0.25*ln(-2*pg)) = d2**0.25 straight off PSUM;
                # sims = exp(-c*dist).  Only the (masked-out) diagonal can
                # go NaN -- off-diagonal d2 ~ 2000 > 0.
                lt = wpool.tile([128, 128], f32, name="lt")
                nc.scalar.activation(lt, pg, AF.Ln, scale=-2.0)
                dist = wpool.tile([128, 128], f32, name="dist")
                nc.scalar.activation(dist, lt, AF.Exp, scale=0.25)
                # mask BEFORE the last Exp: s >= t (incl the NaN diagonal)
                # gets a distance big enough that exp(-c*that) == 0, so the
                # final ACT writes the masked bf16 sims directly and the
                # gpsimd op is off the numers critical path.
                distM = wpool.tile([128, 128], f32, name="distM")
                nc.gpsimd.affine_select(
                    out=distM, in_=dist,
                    compare_op=OP.is_ge, fill=max(40.0, 40.0 / abs(cval)),
                    base=-1, pattern=[[1, 128]], channel_multiplier=-1,
                )
                simsM = smpool.tile([128, 128], bf16, name="simsM")
                nc.scalar.activation(simsM, distM, AF.Exp, scale=float(-cval))
                sims_of[b] = simsM

            def epiB(b, c):
                # numers[t, cls] = sum_s simsM[s,t] * teach[s, cls]
                pn = pnpool.tile([128, NC], f32, name="pn")
                nc.tensor.matmul(
                    pn, sims_of.pop(b), teach_sb[:, b, :],
                    start=True, stop=True,
                )
                osl = scb_all[:, b * NC:(b + 1) * NC]
                rden = spool.tile([128, 1], f32, name="rden")
                if gamma_is_one:
                    # score = (numers+eps) / sum_cls(numers+eps): pure DVE.
                    tmp = spool.tile([128, NC], f32, name="tmp")
                    den = spool.tile([128, 1], f32, name="den")
                    nc.vector.tensor_scalar(
                        tmp, pn, EPS_NUMER, 0.0, op0=OP.add, op1=OP.add,
                        accum_out=den,
                    )
                    nc.vector.reciprocal(rden, den)
                    nc.vector.tensor_scalar(osl, tmp, rden, None, op0=OP.mult)
                else:
                    # tmp = (numers + eps) ** gamma  via exp(gamma * ln(.)).
                    l2 = spool.tile([128, NC], f32, name="l2")
                    nc.scalar.activation(l2, pn, AF.Ln, bias=eps_sb)
                    tmp = spool.tile([128, NC], f32, name="tmp")
                    nc.scalar.activation(tmp, l2, AF.Exp, scale=gam)
                    den = spool.tile([128, 1], f32, name="den")
                    nc.vector.tensor_reduce(
                        den, tmp, axis=mybir.AxisListType.X, op=OP.add,
                    )
                    nc.vector.reciprocal(rden, den)
                    nc.vector.tensor_scalar(osl, tmp, rden, None, op0=OP.mult)

            # ---- schedule: epilogues of chunk c-1 spread through the
            # d-tile slots of chunk c: A1 at slots 1..4, A2 one slot later
            # (covers the gpsimd diag latency), B at 4..7, T at the end.
            seq0 = [sum(NSEQ[:i]) for i in range(len(CHUNKS))]
            for c in range(len(CHUNKS)):
                for dt_i in range(DT):
                    feats_dt(c, dt_i)
                    if c > 0:
                        b0, n = seq0[c - 1], NSEQ[c - 1]
                        if 1 <= dt_i <= n:
                            epiA1(b0 + dt_i - 1)
                        if 2 <= dt_i <= n + 1:
                            epiA2(b0 + dt_i - 2)
                        if 4 <= dt_i <= n + 3:
                            epiB(b0 + dt_i - 4, c - 1)
            # last chunk: issue Gram groups back-to-back (3 PSUM bufs)
            # so the PE chews Gram matmuls while gpsimd extracts diagonals;
            # A1(3) is deferred until Ln(0) has freed pg(0)'s bank.
            cl = len(CHUNKS) - 1
            b0, n = seq0[cl], NSEQ[cl]
            epiA1(b0 + 0)
            epiA2(b0 + 0)
            epiA1(b0 + 1)
            epiA2(b0 + 1)
            epiB(b0 + 0, cl)
            epiA1(b0 + 2)
            epiA2(b0 + 2)
            epiB(b0 + 1, cl)
            epiA1(b0 + 3)
            epiA2(b0 + 3)
            epiB(b0 + 2, cl)
            epiB(b0 + 3, cl)
            nc.sync.dma_start(out=y_h[:, :], in_=scb_all)

    nc.compile()
    return nc


def _get_bass(gamma_is_one=True, cval=1.0):
    key = ("nc", bool(gamma_is_one), float(cval))
    if key not in _NC_CACHE:
        _NC_CACHE[key] = _build_bass(bool(gamma_is_one), float(cval))
    return _NC_CACHE[key]


def make_in_maps(data_t, teaching_signal_t, W_fe, b_fe, c, gamma):
    """Host-side prep: cast to fp8/bf16, permute for contiguous DMAs, shard."""
    import concourse.mybir as mybir
    mmdt = mybir.dt.np(mybir.dt.float8e4)
    x8 = np.asarray(data_t, np.float32).reshape(B * T, CHW).astype(mmdt)
    W8 = np.asarray(W_fe, np.float32).astype(mmdt)
    # W: [kt*128+p, dt*128+m] -> [p][dt][kt*128+m]
    Wh = np.ascontiguousarray(
        W8.reshape(KT, 128, DT, 128).transpose(1, 2, 0, 3)
    ).reshape(128, DT * KT * 128)
    bfe_pd = np.asarray(b_fe, np.float32).reshape(DT, 128).T     # [128, DT]
    cval = np.float32(np.asarray(c, np.float32).reshape(-1)[0])
    gval = np.float32(np.asarray(gamma, np.float32).reshape(-1)[0])
    pars = np.empty((128, 2 + DT), np.float32)
    pars[:, 0] = -cval
    pars[:, 1] = gval
    pars[:, 2:] = bfe_pd
    teach16 = np.asarray(teaching_signal_t, np.float32).astype(
        ml_dtypes.bfloat16)

    in_maps = []
    for core in range(NCORES):
        rows = slice(core * TOK, (core + 1) * TOK)
        xt = x8[rows].T                               # [CHW, TOK]
        m = dict(Wh=Wh, pars=pars)
        for ci, w in enumerate(CHUNKS):
            # x^T[kt*128+p, C0+j] -> [p][kt*w+j]
            xc = xt[:, C0[ci]:C0[ci] + w].reshape(KT, 128, w)
            m[f"xh{ci}"] = np.ascontiguousarray(
                xc.transpose(1, 0, 2)).reshape(128, KT * w)
        m["teach"] = np.ascontiguousarray(
            teach16[core * BL:(core + 1) * BL].transpose(1, 0, 2)
        ).reshape(T, BL * NC)
        in_maps.append(m)
    return in_maps


def kernel(responses_t, data_t, teaching_signal_t, W_fe, b_fe, c, gamma):
    global LAST_RESULTS
    from concourse.bass_utils import run_bass_kernel_spmd

    in_maps = make_in_maps(data_t, teaching_signal_t, W_fe, b_fe, c, gamma)
    gval = float(np.asarray(gamma, np.float32).reshape(-1)[0])
    cval = float(np.asarray(c, np.float32).reshape(-1)[0])
    nc = _get_bass(gamma_is_one=(gval == 1.0), cval=cval)
    res = run_bass_kernel_spmd(nc, in_maps, core_ids=list(range(NCORES)))
    LAST_RESULTS = res
    y = np.concatenate(
        [r["yT2"].reshape(T, BL, NC).transpose(1, 0, 2)
         for r in res.results], axis=0)
    y[:, 0, :] = EPS_NUMER                  # t == 0 rows, as in the reference
    return np.ascontiguousarray(y[:, :, None, :].astype(np.float32))


# revision 40
# speedup vs baseline: 1.0012x; 1.0012x over previous
"""Trainium2 Bass kernel for the ExemplarBaseline retrieval-kNN model.

Math (per batch b, fully independent across b):
    f      = data.reshape(B*T, CHW) @ W_fe + b_fe            (feature extract)
    d2     = ||f_s - f_t||^2 ; dist = d2**0.25
    sims   = exp(-c * dist)
    numers = 1e-8 + sum_{s<t} sims[s,t] * teach[s, cls]
    score  = numers**gamma / sum_cls ; score[t=0] = 1e-8

Sharding: data-parallel over the batch dim B (128) across 8 NeuronCores,
16 sequences per core (BL=16, T=128 -> TOK=2048 tokens per core).

Structure (v4):
  - Host pre-permutes x/W/teach so every DMA descriptor is a 1.5KB+
    contiguous run; ALL inputs (x 48KB/partition fp8 + W 24KB) live in
    SBUF, DMA'd up front in pieces ordered by DEADLINE across BOTH
    HWDGE trigger engines (sync + scalar) so the critical ~2MB for the
    first chunk's dt0 drains first.
  - feats^T = W^T x^T in fp8+DoubleRow (2x PE rate), evacuated with
    bias-add directly to fp8 fT pair tiles [128, 2, TOK].
  - d2 = sq_s + sq_t - 2*Gram, built ENTIRELY inside one PSUM group per
    sequence: 4 fp8-DoubleRow Gram matmuls, then sq is read off the
    Gram DIAGONAL (DVE copy to bf16, gpsimd affine_select is_equal ->
    diag tile) and both rank-1 corrections are two bf16 matmuls against
    a -0.5 constant tile (stat=diag/mov=-.5 adds -0.5*sq_s;
    stat=-.5/mov=diag adds -0.5*sq_t).  This kills the old per-chunk sq
    machinery (f2 muls on DVE, ones-matmul on PE, serial hi/lo fp8
    chain) and is MORE precise (bf16 sq vs fp8 hi/lo pair).
  - sims = exp(-c * exp(0.25 * ln(-2*pg))) -- Ln/Exp share one ACT
    table set (patched chooser), no table reloads anywhere; c is baked
    into the compiled kernel as an immediate scale (AP-scale ACTs cost
    ~90ns extra each).
  - gamma == 1 fast path (the reference setup fills gamma with ones):
    score = (numers+eps)/sum(numers+eps) on DVE only -- no Ln/Exp in
    the class normalizer.  A general-gamma variant is compiled instead
    when the host sees gamma != 1.
  - the triangular mask runs on DIST (gpsimd affine_select, fill big
    enough that exp(-c*fill)==0), so the final Exp writes the masked
    bf16 sims directly and gpsimd is off the numers critical path.
  - token chunks 4 x 512; epilogue split epiA1 (Gram+diag) / epiA2
    (rank1s + ACT chain) / epiB (numers+norm) spread through the NEXT
    chunk's d-tile slots; the LAST chunk's dt7 is evacuated per
    sequence and its ladders issue ACT-first (the tail is ACT-bound,
    so the PE waits on the diag extract instead of the reverse).
  - scores staged in one persistent [128, 160] f32 tile (partition=t),
    ONE output DMA; the host does the tiny [T,BL,NC]->[BL,T,NC]
    transpose and the t==0 EPS override.
Error budget: rel err ~9.5e-3 measured vs the 2e-2 gate (fp8 feats
dominates; fp8 Gram + bf16 sq/sims/teach add the rest).
"""

import numpy as np
import ml_dtypes

B, T, NC = 128, 128, 10
CHW, D = 3072, 1024
NCORES = 8
BL = B // NCORES          # 16 sequences per core
TOK = BL * T              # 2048 tokens per core
KT = CHW // 128           # 24 contraction tiles
DT = D // 128             # 8 feature tiles

CHUNKS = [512, 512, 512, 512]          # token columns per chunk
NSEQ = [w // T for w in CHUNKS]        # sequences per chunk [4,4,4,4]
C0 = [sum(CHUNKS[:i]) for i in range(len(CHUNKS))]   # chunk col starts

EPS_NUMER = 1e-8

_NC_CACHE = {}
LAST_RESULTS = None       # BassKernelResults of the most recent run (for test.py)


def _build_bass(gamma_is_one, cval):
    import concourse.mybir as mybir
    import concourse.tile as tile
    from concourse import bacc

    f32 = mybir.dt.float32
    bf16 = mybir.dt.bfloat16
    fp8 = mybir.dt.float8e4
    AF = mybir.ActivationFunctionType
    OP = mybir.AluOpType
    PM = mybir.MatmulPerfMode

    # The ACT table-set chooser picks the FIRST set containing each function:
    # Exp -> set 0, Ln -> set 5, which makes every Ln<->Exp transition reload
    # tables (~1.3us each).  Both live together in natural_log_exp_and_others;
    # hide them from every other set so the chooser lands there once.
    if not getattr(bacc, "_ln_exp_tables_patched", False):
        orig_tables = bacc.get_activation_tables

        def _patched_tables(arch):
            out = {}
            for name, funcs in orig_tables(arch).items():
                if name != "natural_log_exp_and_others":
                    funcs = funcs - {AF.Ln, AF.Exp}
                out[name] = funcs
            return out

        bacc.get_activation_tables = _patched_tables
        bacc._ln_exp_tables_patched = True

    nc = bacc.Bacc("TRN2", target_bir_lowering=False)

    # Host-side layouts (see make_in_maps): per-chunk x tensors and
    # dt-major W so every DMA slice is contiguous per partition.
    x_h = [
        nc.dram_tensor(f"xh{c}", [128, KT * w], fp8, kind="ExternalInput")
        for c, w in enumerate(CHUNKS)
    ]
    W_h = nc.dram_tensor("Wh", [128, DT * KT * 128], fp8, kind="ExternalInput")
    teach_h = nc.dram_tensor("teach", [T, BL * NC], bf16, kind="ExternalInput")
    pars_h = nc.dram_tensor("pars", [128, 2 + DT], f32, kind="ExternalInput")
    y_h = nc.dram_tensor("yT2", [T, BL * NC], f32, kind="ExternalOutput")

    with tile.TileContext(nc) as tc:
        with (
            tc.tile_pool(name="cpool", bufs=1) as cpool,
            tc.tile_pool(name="wpool", bufs=4) as wpool,
            tc.tile_pool(name="dtlpool", bufs=3) as dtlpool,
            tc.tile_pool(name="smpool", bufs=3) as smpool,
            tc.tile_pool(name="spool", bufs=6) as spool,
            tc.tile_pool(name="pfpool", bufs=2, space="PSUM") as pfpool,
            tc.tile_pool(name="pgpool", bufs=3, space="PSUM") as pgpool,
            tc.tile_pool(name="pnpool", bufs=1, space="PSUM") as pnpool,
        ):
            # ---- persistent tiles -------------------------------------
            W_sb = cpool.tile([128, DT, KT, 128], fp8, name="W_sb")
            x_sb = [
                cpool.tile([128, KT, w], fp8, name=f"x_sb{c}")
                for c, w in enumerate(CHUNKS)
            ]
            teach_sb = cpool.tile([128, BL, NC], bf16, name="teach_sb")
            pars_sb = cpool.tile([128, 2 + DT], f32, name="pars_sb")
            eps_sb = cpool.tile([128, 1], f32, name="eps_sb")
            # all scores staged here (partition = t), ONE output DMA at
            # the end; the host does the final [T,BL,NC]->[BL,T,NC]
            # transpose and the t==0 EPS override
            scb_all = cpool.tile([128, BL * NC], f32, name="scb_all")
            # fT in fp8 DoubleRow pair layout: tile p holds d-tiles 2p, 2p+1
            fTp = [
                cpool.tile([128, 2, TOK], fp8, name=f"fTp{i}")
                for i in range(DT // 2)
            ]
            # constants for the rank-1 sq corrections: ones plane and a
            # diagonal(-0.5) so diag extraction is ONE DVE multiply
            ones_pl = cpool.tile([128, 128], bf16, name="ones_pl")
            eyehalf = cpool.tile([128, 128], bf16, name="eyehalf")
            gam = pars_sb[:, 1:2]

            # ---- all input DMAs, deadline order -----------------------
            # Each dma_start costs ~0.62us of DIRECT2D descriptor-writing
            # on its issuing sequencer and each sequencer's ring holds only
            # 4 in-flight pieces, so the critical pieces (x0 + W dt0, the
            # ~2MB chunk-0/dt0 working set) lead BOTH trigger engines.
            def xpiece(eng, c, k0, k1):
                eng.dma_start(
                    out=x_sb[c][:, k0:k1, :],
                    in_=x_h[c][:, k0 * CHUNKS[c]:k1 * CHUNKS[c]],
                )

            def wpiece(eng, dt_i, k0, k1):
                KW = KT * 128
                eng.dma_start(
                    out=W_sb[:, dt_i, k0:k1, :],
                    in_=W_h[:, dt_i * KW + k0 * 128:dt_i * KW + k1 * 128],
                )

            # sync: x0 k0:12 in 4 pieces, then W dt2..7, teach, x2
            for k in range(0, 12, 3):                  # x0 p1-p4
                xpiece(nc.sync, 0, k, k + 3)
            for dt_i in range(2, DT):                  # W dt2..7
                wpiece(nc.sync, dt_i, 0, KT)
            nc.sync.dma_start(out=teach_sb, in_=teach_h[:, :])
            for k in range(0, KT, 6):                  # x2: 4 pieces
                xpiece(nc.sync, 2, k, k + 6)
            # scalar (ACT): W dt0 + x0 tail interleaved, pars early (it
            # gates the very first feats evacuation), then W dt1, x1, x3
            wpiece(nc.scalar, 0, 0, 12)                # W dt0 p1
            xpiece(nc.scalar, 0, 12, 18)               # x0 p5
            wpiece(nc.scalar, 0, 12, KT)               # W dt0 p2
            nc.scalar.dma_start(out=pars_sb, in_=pars_h[:, :])
            xpiece(nc.scalar, 0, 18, KT)               # x0 p6
            wpiece(nc.scalar, 1, 0, 12)                # W dt1 p1
            wpiece(nc.scalar, 1, 12, KT)               # W dt1 p2
            for k in range(0, KT, 6):                  # x1: 4 pieces
                xpiece(nc.scalar, 1, k, k + 6)
            for k in range(0, KT, 6):                  # x3: 4 pieces
                xpiece(nc.scalar, 3, k, k + 6)

            # constants: few-partition memsets are slow on DVE,
            # gpsimd is idle at startup
            nc.gpsimd.memset(ones_pl, -0.5)
            nc.gpsimd.affine_select(
                out=eyehalf, in_=ones_pl,
                compare_op=OP.is_equal, fill=0.0,
                base=0, pattern=[[1, 128]], channel_multiplier=-1,
            )
            nc.gpsimd.memset(ones_pl, 1.0)
            nc.vector.memset(eps_sb, EPS_NUMER)

            # ---- per-(chunk, d-tile) feats ----------------------------
            def feats_dt(c, dt_i):
                w = CHUNKS[c]
                csl = slice(C0[c], C0[c] + w)
                pf = pfpool.tile([128, w], f32, name="pf")
                for k in range(0, KT, 2):
                    nc.tensor.matmul(
                        pf, W_sb[:, dt_i, k:k + 2, :], x_sb[c][:, k:k + 2, :],
                        start=(k == 0), stop=(k == KT - 2),
                        perf_mode=PM.DoubleRow,
                    )
                # evacuate psum -> fp8 fT pair tile with per-partition bias
                # add.  On DVE so the scalar engine only ever runs Ln/Exp.
                pair = fTp[dt_i // 2][:, dt_i % 2, :]
                if c == len(CHUNKS) - 1 and dt_i == DT - 1:
                    # last chunk's last dt: evacuate per SEQUENCE so each
                    # Gram group starts after only its own 128 columns.
                    for i in range(NSEQ[c]):
                        nc.vector.tensor_scalar(
                            pair[:, C0[c] + i * T:C0[c] + (i + 1) * T],
                            pf[:, i * T:(i + 1) * T],
                            pars_sb[:, 2 + dt_i:3 + dt_i], None, op0=OP.add,
                        )
                else:
                    nc.vector.tensor_scalar(
                        pair[:, csl], pf,
                        pars_sb[:, 2 + dt_i:3 + dt_i], None, op0=OP.add,
                    )

            # ---- per-sequence epilogue, split A1/A2/B ------------------
            pg_of, dtl_of, sims_of = {}, {}, {}

            def epiA1(b):
                # Gram accumulation (group left OPEN) + diagonal extract:
                # diag(G)_ss = sq_s, pulled by gpsimd off an SBUF copy.
                tsl = slice(b * T, (b + 1) * T)
                pg = pgpool.tile([128, 128], f32, name="pg")
                for p in range(DT // 2):
                    nc.tensor.matmul(
                        pg, fTp[p][:, :, tsl], fTp[p][:, :, tsl],
                        start=(p == 0), stop=False, perf_mode=PM.DoubleRow,
                    )
                # dtl = diag(-0.5 * sq) in ONE DVE op: pg * diag(-0.5)
                dtl = dtlpool.tile([128, 128], bf16, name="dtl")
                nc.vector.tensor_mul(dtl, pg, eyehalf)
                pg_of[b], dtl_of[b] = pg, dtl

            def epiA2(b):
                # rank-1 corrections off the diag tile, then the ACT chain.
                pg, dtl = pg_of.pop(b), dtl_of.pop(b)
                # stat=diag(-.5sq), mov=ones: pg[s,t] += -0.5*sq_s
                nc.tensor.matmul(pg, dtl, ones_pl, start=False, stop=False)
                # stat=ones, mov=diag(-.5sq): pg[s,t] += -0.5*sq_t
                nc.tensor.matmul(pg, ones_pl, dtl, start=False, stop=True)
                # dist = exp(0.25*ln(-2*pg)) = d2**0.25 straight off PSUM;
                # sims = exp(-c*dist).  Only the (masked-out) diagonal can
                # go NaN -- off-diagonal d2 ~ 2000 > 0.
                lt = wpool.tile([128, 128], f32, name="lt")
                nc.scalar.activation(lt, pg, AF.Ln, scale=-2.0)
                dist = wpool.tile([128, 128], f32, name="dist")
                nc.scalar.activation(dist, lt, AF.Exp, scale=0.25)
                # mask BEFORE the last Exp: s >= t (incl the NaN diagonal)
                # gets a distance big enough that exp(-c*that) == 0, so the
                # final ACT writes the masked bf16 sims directly and the
                # gpsimd op is off the numers critical path.
                distM = wpool.tile([128, 128], f32, name="distM")
                nc.gpsimd.affine_select(
                    out=distM, in_=dist,
                    compare_op=OP.is_ge, fill=max(40.0, 40.0 / abs(cval)),
                    base=-1, pattern=[[1, 128]], channel_multiplier=-1,
                )
                simsM = smpool.tile([128, 128], bf16, name="simsM")
                nc.scalar.activation(simsM, distM, AF.Exp, scale=float(-cval))
                sims_of[b] = simsM

            def epiB(b, c):
                # numers[t, cls] = sum_s simsM[s,t] * teach[s, cls]
                pn = pnpool.tile([128, NC], f32, name="pn")
                nc.tensor.matmul(
                    pn, sims_of.pop(b), teach_sb[:, b, :],
                    start=True, stop=True,
                )
                osl = scb_all[:, b * NC:(b + 1) * NC]
                rden = spool.tile([128, 1], f32, name="rden")
                if gamma_is_one:
                    # score = (numers+eps) / sum_cls(numers+eps): pure DVE.
                    tmp = spool.tile([128, NC], f32, name="tmp")
                    den = spool.tile([128, 1], f32, name="den")
                    nc.vector.tensor_scalar(
                        tmp, pn, EPS_NUMER, 0.0, op0=OP.add, op1=OP.add,
                        accum_out=den,
                    )
                    nc.vector.reciprocal(rden, den)
                    nc.vector.tensor_scalar(osl, tmp, rden, None, op0=OP.mult)
                else:
                    # tmp = (numers + eps) ** gamma  via exp(gamma * ln(.)).
                    l2 = spool.tile([128, NC], f32, name="l2")
                    nc.scalar.activation(l2, pn, AF.Ln, bias=eps_sb)
                    tmp = spool.tile([128, NC], f32, name="tmp")
                    nc.scalar.activation(tmp, l2, AF.Exp, scale=gam)
                    den = spool.tile([128, 1], f32, name="den")
                    nc.vector.tensor_reduce(
                        den, tmp, axis=mybir.AxisListType.X, op=OP.add,
                    )
                    nc.vector.reciprocal(rden, den)
                    nc.vector.tensor_scalar(osl, tmp, rden, None, op0=OP.mult)

            # ---- schedule: epilogues of chunk c-1 spread through the
            # d-tile slots of chunk c: A1 at slots 1..4, A2 one slot later
            # (covers the gpsimd diag latency), B at 4..7, T at the end.
            seq0 = [sum(NSEQ[:i]) for i in range(len(CHUNKS))]
            for c in range(len(CHUNKS)):
                for dt_i in range(DT):
                    feats_dt(c, dt_i)
                    if c > 0:
                        b0, n = seq0[c - 1], NSEQ[c - 1]
                        if 1 <= dt_i <= n:
                            epiA1(b0 + dt_i - 1)
                        if 2 <= dt_i <= n + 1:
                            epiA2(b0 + dt_i - 2)
                        if 4 <= dt_i <= n + 3:
                            epiB(b0 + dt_i - 4, c - 1)
            # last chunk: issue Gram groups back-to-back (3 PSUM bufs)
            # so the PE chews Gram matmuls while gpsimd extracts diagonals;
            # A1(3) is deferred until Ln(0) has freed pg(0)'s bank.
            cl = len(CHUNKS) - 1
            b0, n = seq0[cl], NSEQ[cl]
            epiA1(b0 + 0)
            epiA2(b0 + 0)
            epiA1(b0 + 1)
            epiA2(b0 + 1)
            epiB(b0 + 0, cl)
            epiA1(b0 + 2)
            epiA2(b0 + 2)
            epiB(b0 + 1, cl)
            epiA1(b0 + 3)
            epiA2(b0 + 3)
            epiB(b0 + 2, cl)
            epiB(b0 + 3, cl)
            nc.sync.dma_start(out=y_h[:, :], in_=scb_all)

    nc.compile()
    return nc


def _get_bass(gamma_is_one=True, cval=1.0):
    key = ("nc", bool(gamma_is_one), float(cval))
    if key not in _NC_CACHE:
        _NC_CACHE[key] = _build_bass(bool(gamma_is_one), float(cval))
    return _NC_CACHE[key]


def make_in_maps(data_t, teaching_signal_t, W_fe, b_fe, c, gamma):
    """Host-side prep: cast to fp8/bf16, permute for contiguous DMAs, shard."""
    import concourse.mybir as mybir
    mmdt = mybir.dt.np(mybir.dt.float8e4)
    x8 = np.asarray(data_t, np.float32).reshape(B * T, CHW).astype(mmdt)
    W8 = np.asarray(W_fe, np.float32).astype(mmdt)
    # W: [kt*128+p, dt*128+m] -> [p][dt][kt*128+m]
    Wh = np.ascontiguousarray(
        W8.reshape(KT, 128, DT, 128).transpose(1, 2, 0, 3)
    ).reshape(128, DT * KT * 128)
    bfe_pd = np.asarray(b_fe, np.float32).reshape(DT, 128).T     # [128, DT]
    cval = np.float32(np.asarray(c, np.float32).reshape(-1)[0])
    gval = np.float32(np.asarray(gamma, np.float32).reshape(-1)[0])
    pars = np.empty((128, 2 + DT), np.float32)
    pars[:, 0] = -cval
    pars[:, 1] = gval
    pars[:, 2:] = bfe_pd
    teach16 = np.asarray(teaching_signal_t, np.float32).astype(
        ml_dtypes.bfloat16)

    in_maps = []
    for core in range(NCORES):
        rows = slice(core * TOK, (core + 1) * TOK)
        xt = x8[rows].T                               # [CHW, TOK]
        m = dict(Wh=Wh, pars=pars)
        for ci, w in enumerate(CHUNKS):
            # x^T[kt*128+p, C0+j] -> [p][kt*w+j]
            xc = xt[:, C0[ci]:C0[ci] + w].reshape(KT, 128, w)
            m[f"xh{ci}"] = np.ascontiguousarray(
                xc.transpose(1, 0, 2)).reshape(128, KT * w)
        m["teach"] = np.ascontiguousarray(
            teach16[core * BL:(core + 1) * BL].transpose(1, 0, 2)
        ).reshape(T, BL * NC)
        in_maps.append(m)
    return in_maps


def kernel(responses_t, data_t, teaching_signal_t, W_fe, b_fe, c, gamma):
    global LAST_RESULTS
    from concourse.bass_utils import run_bass_kernel_spmd

    in_maps = make_in_maps(data_t, teaching_signal_t, W_fe, b_fe, c, gamma)
    gval = float(np.asarray(gamma, np.float32).reshape(-1)[0])
    cval = float(np.asarray(c, np.float32).reshape(-1)[0])
    nc = _get_bass(gamma_is_one=(gval == 1.0), cval=cval)
    res = run_bass_kernel_spmd(nc, in_maps, core_ids=list(range(NCORES)))
    LAST_RESULTS = res
    y = np.concatenate(
        [r["yT2"].reshape(T, BL, NC).transpose(1, 0, 2)
         for r in res.results], axis=0)
    y[:, 0, :] = EPS_NUMER                  # t == 0 rows, as in the reference
    return np.ascontiguousarray(y[:, :, None, :].astype(np.float32))


# revision 41
# speedup vs baseline: 1.0086x; 1.0073x over previous
"""Trainium2 Bass kernel for the ExemplarBaseline retrieval-kNN model.

Math (per batch b, fully independent across b):
    f      = data.reshape(B*T, CHW) @ W_fe + b_fe            (feature extract)
    d2     = ||f_s - f_t||^2 ; dist = d2**0.25
    sims   = exp(-c * dist)
    numers = 1e-8 + sum_{s<t} sims[s,t] * teach[s, cls]
    score  = numers**gamma / sum_cls ; score[t=0] = 1e-8

Sharding: data-parallel over the batch dim B (128) across 8 NeuronCores,
16 sequences per core (BL=16, T=128 -> TOK=2048 tokens per core).

Structure (v4):
  - Host pre-permutes x/W/teach so every DMA descriptor is a 1.5KB+
    contiguous run; ALL inputs (x 48KB/partition fp8 + W 24KB) live in
    SBUF, DMA'd up front in pieces ordered by DEADLINE across BOTH
    HWDGE trigger engines (sync + scalar) so the critical ~2MB for the
    first chunk's dt0 drains first.
  - feats^T = W^T x^T in fp8+DoubleRow (2x PE rate), evacuated with
    bias-add directly to fp8 fT pair tiles [128, 2, TOK].
  - d2 = sq_s + sq_t - 2*Gram, built ENTIRELY inside one PSUM group per
    sequence: 4 fp8-DoubleRow Gram matmuls, then -0.5*sq is read off
    the Gram DIAGONAL in ONE DVE multiply (pg * diag(-0.5) constant ->
    bf16 diag tile) and both rank-1 corrections are two bf16 matmuls
    against a ones plane (stat=diag/mov=ones adds -0.5*sq_s;
    stat=ones/mov=diag adds -0.5*sq_t).  This kills the old per-chunk
    sq machinery (f2 muls on DVE, ones-matmul on PE, serial hi/lo fp8
    chain) and is MORE precise (bf16 sq vs fp8 hi/lo pair).
  - sims = exp(-c * exp(0.25 * ln(-2*pg))) -- Ln/Exp share one ACT
    table set (patched chooser), no table reloads anywhere; c is baked
    into the compiled kernel as an immediate scale (AP-scale ACTs cost
    ~90ns extra each).
  - gamma == 1 fast path (the reference setup fills gamma with ones):
    score = (numers+eps)/sum(numers+eps) on DVE only -- no Ln/Exp in
    the class normalizer.  A general-gamma variant is compiled instead
    when the host sees gamma != 1.
  - the triangular mask runs on DIST (gpsimd affine_select, fill big
    enough that exp(-c*fill)==0), so the final Exp writes the masked
    bf16 sims directly and gpsimd is off the numers critical path.
  - token chunks 4 x 512; epilogue split epiA1 (Gram+diag) / epiA2
    (rank1s + ACT chain) / epiB (numers+norm) spread through the NEXT
    chunk's d-tile slots; the LAST chunk's dt7 is evacuated per
    sequence and its ladders issue ACT-first (the tail is ACT-bound,
    so the PE waits on the diag extract instead of the reverse).
  - scores staged in one persistent [128, 160] f32 tile (partition=t),
    ONE output DMA; the host does the tiny [T,BL,NC]->[BL,T,NC]
    transpose and the t==0 EPS override.
Error budget: rel err ~9.5e-3 measured vs the 2e-2 gate (fp8 feats
dominates; fp8 Gram + bf16 sq/sims/teach add the rest).
"""

import numpy as np
import ml_dtypes

B, T, NC = 128, 128, 10
CHW, D = 3072, 1024
NCORES = 8
BL = B // NCORES          # 16 sequences per core
TOK = BL * T              # 2048 tokens per core
KT = CHW // 128           # 24 contraction tiles
DT = D // 128             # 8 feature tiles

CHUNKS = [512, 512, 512, 512]          # token columns per chunk
NSEQ = [w // T for w in CHUNKS]        # sequences per chunk [4,4,4,4]
C0 = [sum(CHUNKS[:i]) for i in range(len(CHUNKS))]   # chunk col starts

EPS_NUMER = 1e-8

_NC_CACHE = {}
LAST_RESULTS = None       # BassKernelResults of the most recent run (for test.py)


def _build_bass(gamma_is_one, cval):
    import concourse.mybir as mybir
    import concourse.tile as tile
    from concourse import bacc

    f32 = mybir.dt.float32
    bf16 = mybir.dt.bfloat16
    fp8 = mybir.dt.float8e4
    AF = mybir.ActivationFunctionType
    OP = mybir.AluOpType
    PM = mybir.MatmulPerfMode

    # The ACT table-set chooser picks the FIRST set containing each function:
    # Exp -> set 0, Ln -> set 5, which makes every Ln<->Exp transition reload
    # tables (~1.3us each).  Both live together in natural_log_exp_and_others;
    # hide them from every other set so the chooser lands there once.
    if not getattr(bacc, "_ln_exp_tables_patched", False):
        orig_tables = bacc.get_activation_tables

        def _patched_tables(arch):
            out = {}
            for name, funcs in orig_tables(arch).items():
                if name != "natural_log_exp_and_others":
                    funcs = funcs - {AF.Ln, AF.Exp}
                out[name] = funcs
            return out

        bacc.get_activation_tables = _patched_tables
        bacc._ln_exp_tables_patched = True

    nc = bacc.Bacc("TRN2", target_bir_lowering=False)

    # Host-side layouts (see make_in_maps): per-chunk x tensors and
    # dt-major W so every DMA slice is contiguous per partition.
    x_h = [
        nc.dram_tensor(f"xh{c}", [128, KT * w], fp8, kind="ExternalInput")
        for c, w in enumerate(CHUNKS)
    ]
    W_h = nc.dram_tensor("Wh", [128, DT * KT * 128], fp8, kind="ExternalInput")
    teach_h = nc.dram_tensor("teach", [T, BL * NC], bf16, kind="ExternalInput")
    pars_h = nc.dram_tensor("pars", [128, 2 + DT], f32, kind="ExternalInput")
    y_h = nc.dram_tensor("yT2", [T, BL * NC], f32, kind="ExternalOutput")

    with tile.TileContext(nc) as tc:
        with (
            tc.tile_pool(name="cpool", bufs=1) as cpool,
            tc.tile_pool(name="wpool", bufs=4) as wpool,
            tc.tile_pool(name="dtlpool", bufs=3) as dtlpool,
            tc.tile_pool(name="smpool", bufs=3) as smpool,
            tc.tile_pool(name="spool", bufs=6) as spool,
            tc.tile_pool(name="pfpool", bufs=2, space="PSUM") as pfpool,
            tc.tile_pool(name="pgpool", bufs=3, space="PSUM") as pgpool,
            tc.tile_pool(name="pnpool", bufs=1, space="PSUM") as pnpool,
        ):
            # ---- persistent tiles -------------------------------------
            W_sb = cpool.tile([128, DT, KT, 128], fp8, name="W_sb")
            x_sb = [
                cpool.tile([128, KT, w], fp8, name=f"x_sb{c}")
                for c, w in enumerate(CHUNKS)
            ]
            teach_sb = cpool.tile([128, BL, NC], bf16, name="teach_sb")
            pars_sb = cpool.tile([128, 2 + DT], f32, name="pars_sb")
            eps_sb = cpool.tile([128, 1], f32, name="eps_sb")
            # all scores staged here (partition = t), ONE output DMA at
            # the end; the host does the final [T,BL,NC]->[BL,T,NC]
            # transpose and the t==0 EPS override
            scb_all = cpool.tile([128, BL * NC], f32, name="scb_all")
            # fT in fp8 DoubleRow pair layout: tile p holds d-tiles 2p, 2p+1
            fTp = [
                cpool.tile([128, 2, TOK], fp8, name=f"fTp{i}")
                for i in range(DT // 2)
            ]
            # constants for the rank-1 sq corrections: ones plane and a
            # diagonal(-0.5) so diag extraction is ONE DVE multiply
            ones_pl = cpool.tile([128, 128], bf16, name="ones_pl")
            eyehalf = cpool.tile([128, 128], bf16, name="eyehalf")
            gam = pars_sb[:, 1:2]

            # ---- all input DMAs, deadline order -----------------------
            # Each dma_start costs ~0.62us of DIRECT2D descriptor-writing
            # on its issuing sequencer and each sequencer's ring holds only
            # 4 in-flight pieces, so the critical pieces (x0 + W dt0, the
            # ~2MB chunk-0/dt0 working set) lead BOTH trigger engines.
            def xpiece(eng, c, k0, k1):
                eng.dma_start(
                    out=x_sb[c][:, k0:k1, :],
                    in_=x_h[c][:, k0 * CHUNKS[c]:k1 * CHUNKS[c]],
                )

            def wpiece(eng, dt_i, k0, k1):
                KW = KT * 128
                eng.dma_start(
                    out=W_sb[:, dt_i, k0:k1, :],
                    in_=W_h[:, dt_i * KW + k0 * 128:dt_i * KW + k1 * 128],
                )

            # sync: x0 k0:12 in 4 pieces, then W dt2..7, teach, x2
            for k in range(0, 12, 3):                  # x0 p1-p4
                xpiece(nc.sync, 0, k, k + 3)
            for dt_i in range(2, DT):                  # W dt2..7
                wpiece(nc.sync, dt_i, 0, KT)
            nc.sync.dma_start(out=teach_sb, in_=teach_h[:, :])
            for k in range(0, KT, 6):                  # x2: 4 pieces
                xpiece(nc.sync, 2, k, k + 6)
            # scalar (ACT): W dt0 + x0 tail interleaved, pars early (it
            # gates the very first feats evacuation), then W dt1, x1, x3
            wpiece(nc.scalar, 0, 0, 12)                # W dt0 p1
            xpiece(nc.scalar, 0, 12, 18)               # x0 p5
            wpiece(nc.scalar, 0, 12, KT)               # W dt0 p2
            nc.scalar.dma_start(out=pars_sb, in_=pars_h[:, :])
            xpiece(nc.scalar, 0, 18, KT)               # x0 p6
            wpiece(nc.scalar, 1, 0, 12)                # W dt1 p1
            wpiece(nc.scalar, 1, 12, KT)               # W dt1 p2
            for k in range(0, KT, 6):                  # x1: 4 pieces
                xpiece(nc.scalar, 1, k, k + 6)
            for k in range(0, KT, 6):                  # x3: 4 pieces
                xpiece(nc.scalar, 3, k, k + 6)

            # constants: few-partition memsets are slow on DVE,
            # gpsimd is idle at startup
            nc.gpsimd.memset(ones_pl, -0.5)
            nc.gpsimd.affine_select(
                out=eyehalf, in_=ones_pl,
                compare_op=OP.is_equal, fill=0.0,
                base=0, pattern=[[1, 128]], channel_multiplier=-1,
            )
            nc.gpsimd.memset(ones_pl, 1.0)
            nc.vector.memset(eps_sb, EPS_NUMER)

            # ---- per-(chunk, d-tile) feats ----------------------------
            def feats_dt(c, dt_i):
                w = CHUNKS[c]
                csl = slice(C0[c], C0[c] + w)
                pf = pfpool.tile([128, w], f32, name="pf")
                for k in range(0, KT, 2):
                    nc.tensor.matmul(
                        pf, W_sb[:, dt_i, k:k + 2, :], x_sb[c][:, k:k + 2, :],
                        start=(k == 0), stop=(k == KT - 2),
                        perf_mode=PM.DoubleRow,
                    )
                # evacuate psum -> fp8 fT pair tile with per-partition bias
                # add.  On DVE so the scalar engine only ever runs Ln/Exp.
                pair = fTp[dt_i // 2][:, dt_i % 2, :]
                if c == len(CHUNKS) - 1 and dt_i == DT - 1:
                    # last chunk's last dt: evacuate per SEQUENCE so each
                    # Gram group starts after only its own 128 columns.
                    for i in range(NSEQ[c]):
                        nc.vector.tensor_scalar(
                            pair[:, C0[c] + i * T:C0[c] + (i + 1) * T],
                            pf[:, i * T:(i + 1) * T],
                            pars_sb[:, 2 + dt_i:3 + dt_i], None, op0=OP.add,
                        )
                else:
                    nc.vector.tensor_scalar(
                        pair[:, csl], pf,
                        pars_sb[:, 2 + dt_i:3 + dt_i], None, op0=OP.add,
                    )

            # ---- per-sequence epilogue, split A1/A2/B ------------------
            pg_of, dtl_of, sims_of = {}, {}, {}

            def epiA1(b):
                # Gram accumulation (group left OPEN) + diagonal extract:
                # diag(G)_ss = sq_s, pulled by gpsimd off an SBUF copy.
                tsl = slice(b * T, (b + 1) * T)
                pg = pgpool.tile([128, 128], f32, name="pg")
                for p in range(DT // 2):
                    nc.tensor.matmul(
                        pg, fTp[p][:, :, tsl], fTp[p][:, :, tsl],
                        start=(p == 0), stop=False, perf_mode=PM.DoubleRow,
                    )
                # dtl = diag(-0.5 * sq) in ONE DVE op: pg * diag(-0.5)
                dtl = dtlpool.tile([128, 128], bf16, name="dtl")
                nc.vector.tensor_mul(dtl, pg, eyehalf)
                pg_of[b], dtl_of[b] = pg, dtl

            def epiA2(b):
                # rank-1 corrections off the diag tile, then the ACT chain.
                pg, dtl = pg_of.pop(b), dtl_of.pop(b)
                # stat=diag(-.5sq), mov=ones: pg[s,t] += -0.5*sq_s
                nc.tensor.matmul(pg, dtl, ones_pl, start=False, stop=False)
                # stat=ones, mov=diag(-.5sq): pg[s,t] += -0.5*sq_t
                nc.tensor.matmul(pg, ones_pl, dtl, start=False, stop=True)
                # dist = exp(0.25*ln(-2*pg)) = d2**0.25 straight off PSUM;
                # sims = exp(-c*dist).  Only the (masked-out) diagonal can
                # go NaN -- off-diagonal d2 ~ 2000 > 0.
                lt = wpool.tile([128, 128], f32, name="lt")
                nc.scalar.activation(lt, pg, AF.Ln, scale=-2.0)
                dist = wpool.tile([128, 128], f32, name="dist")
                nc.scalar.activation(dist, lt, AF.Exp, scale=0.25)
                # mask BEFORE the last Exp: s >= t (incl the NaN diagonal)
                # gets a distance big enough that exp(-c*that) == 0, so the
                # final ACT writes the masked bf16 sims directly and the
                # gpsimd op is off the numers critical path.
                distM = wpool.tile([128, 128], f32, name="distM")
                nc.gpsimd.affine_select(
                    out=distM, in_=dist,
                    compare_op=OP.is_ge, fill=max(40.0, 40.0 / abs(cval)),
                    base=-1, pattern=[[1, 128]], channel_multiplier=-1,
                )
                simsM = smpool.tile([128, 128], bf16, name="simsM")
                nc.scalar.activation(simsM, distM, AF.Exp, scale=float(-cval))
                sims_of[b] = simsM

            def epiB(b, c):
                # numers[t, cls] = sum_s simsM[s,t] * teach[s, cls]
                pn = pnpool.tile([128, NC], f32, name="pn")
                nc.tensor.matmul(
                    pn, sims_of.pop(b), teach_sb[:, b, :],
                    start=True, stop=True,
                )
                osl = scb_all[:, b * NC:(b + 1) * NC]
                rden = spool.tile([128, 1], f32, name="rden")
                if gamma_is_one:
                    # score = (numers+eps) / sum_cls(numers+eps): pure DVE.
                    tmp = spool.tile([128, NC], f32, name="tmp")
                    den = spool.tile([128, 1], f32, name="den")
                    nc.vector.tensor_scalar(
                        tmp, pn, EPS_NUMER, 0.0, op0=OP.add, op1=OP.add,
                        accum_out=den,
                    )
                    nc.vector.reciprocal(rden, den)
                    nc.vector.tensor_scalar(osl, tmp, rden, None, op0=OP.mult)
                else:
                    # tmp = (numers + eps) ** gamma  via exp(gamma * ln(.)).
                    l2 = spool.tile([128, NC], f32, name="l2")
                    nc.scalar.activation(l2, pn, AF.Ln, bias=eps_sb)
                    tmp = spool.tile([128, NC], f32, name="tmp")
                    nc.scalar.activation(tmp, l2, AF.Exp, scale=gam)
                    den = spool.tile([128, 1], f32, name="den")
                    nc.vector.tensor_reduce(
                        den, tmp, axis=mybir.AxisListType.X, op=OP.add,
                    )
                    nc.vector.reciprocal(rden, den)
                    nc.vector.tensor_scalar(osl, tmp, rden, None, op0=OP.mult)

            # ---- schedule: epilogues of chunk c-1 spread through the
            # d-tile slots of chunk c: A1 at slots 1..4, A2 one slot later
            # (covers the gpsimd diag latency), B at 4..7, T at the end.
            seq0 = [sum(NSEQ[:i]) for i in range(len(CHUNKS))]
            for c in range(len(CHUNKS)):
                for dt_i in range(DT):
                    feats_dt(c, dt_i)
                    if c > 0:
                        b0, n = seq0[c - 1], NSEQ[c - 1]
                        if 1 <= dt_i <= n:
                            epiA1(b0 + dt_i - 1)
                        if 2 <= dt_i <= n + 1:
                            epiA2(b0 + dt_i - 2)
                        if 4 <= dt_i <= n + 3:
                            epiB(b0 + dt_i - 4, c - 1)
            # last chunk: issue Gram groups back-to-back (3 PSUM bufs)
            # so the PE chews Gram matmuls while gpsimd extracts diagonals;
            # A1(3) is deferred until Ln(0) has freed pg(0)'s bank.
            cl = len(CHUNKS) - 1
            b0, n = seq0[cl], NSEQ[cl]
            epiA1(b0 + 0)
            epiA2(b0 + 0)
            epiA1(b0 + 1)
            epiA2(b0 + 1)
            epiB(b0 + 0, cl)
            epiA1(b0 + 2)
            epiA2(b0 + 2)
            epiB(b0 + 1, cl)
            epiA1(b0 + 3)
            epiA2(b0 + 3)
            epiB(b0 + 2, cl)
            epiB(b0 + 3, cl)
            nc.sync.dma_start(out=y_h[:, :], in_=scb_all)

    nc.compile()
    return nc


def _get_bass(gamma_is_one=True, cval=1.0):
    key = ("nc", bool(gamma_is_one), float(cval))
    if key not in _NC_CACHE:
        _NC_CACHE[key] = _build_bass(bool(gamma_is_one), float(cval))
    return _NC_CACHE[key]


def make_in_maps(data_t, teaching_signal_t, W_fe, b_fe, c, gamma):
    """Host-side prep: cast to fp8/bf16, permute for contiguous DMAs, shard."""
    import concourse.mybir as mybir
    mmdt = mybir.dt.np(mybir.dt.float8e4)
    x8 = np.asarray(data_t, np.float32).reshape(B * T, CHW).astype(mmdt)
    W8 = np.asarray(W_fe, np.float32).astype(mmdt)
    # W: [kt*128+p, dt*128+m] -> [p][dt][kt*128+m]
    Wh = np.ascontiguousarray(
        W8.reshape(KT, 128, DT, 128).transpose(1, 2, 0, 3)
    ).reshape(128, DT * KT * 128)
    bfe_pd = np.asarray(b_fe, np.float32).reshape(DT, 128).T     # [128, DT]
    cval = np.float32(np.asarray(c, np.float32).reshape(-1)[0])
    gval = np.float32(np.asarray(gamma, np.float32).reshape(-1)[0])
    pars = np.empty((128, 2 + DT), np.float32)
    pars[:, 0] = -cval
    pars[:, 1] = gval
    pars[:, 2:] = bfe_pd
    teach16 = np.asarray(teaching_signal_t, np.float32).astype(
        ml_dtypes.bfloat16)

    in_maps = []
    for core in range(NCORES):
        rows = slice(core * TOK, (core + 1) * TOK)
        xt = x8[rows].T                               # [CHW, TOK]
        m = dict(Wh=Wh, pars=pars)
        for ci, w in enumerate(CHUNKS):
            # x^T[kt*128+p, C0+j] -> [p][kt*w+j]
            xc = xt[:, C0[ci]:C0[ci] + w].reshape(KT, 128, w)
            m[f"xh{ci}"] = np.ascontiguousarray(
                xc.transpose(1, 0, 2)).reshape(128, KT * w)
        m["teach"] = np.ascontiguousarray(
            teach16[core * BL:(core + 1) * BL].transpose(1, 0, 2)
        ).reshape(T, BL * NC)
        in_maps.append(m)
    return in_maps


def kernel(responses_t, data_t, teaching_signal_t, W_fe, b_fe, c, gamma):
    global LAST_RESULTS
    from concourse.bass_utils import run_bass_kernel_spmd

    in_maps = make_in_maps(data_t, teaching_signal_t, W_fe, b_fe, c, gamma)
    gval = float(np.asarray(gamma, np.float32).reshape(-1)[0])
    cval = float(np.asarray(c, np.float32).reshape(-1)[0])
    nc = _get_bass(gamma_is_one=(gval == 1.0), cval=cval)
    res = run_bass_kernel_spmd(nc, in_maps, core_ids=list(range(NCORES)))
    LAST_RESULTS = res
    y = np.concatenate(
        [r["yT2"].reshape(T, BL, NC).transpose(1, 0, 2)
         for r in res.results], axis=0)
    y[:, 0, :] = EPS_NUMER                  # t == 0 rows, as in the reference
    return np.ascontiguousarray(y[:, :, None, :].astype(np.float32))
